# revision 25
# baseline (speedup 1.0000x reference)
"""Supervised contrastive loss on 8 Trainium2 NeuronCores.

Reference (N=8192, D=128, TAU=0.1, 100 classes):
    xn   = x / ||x||_row
    e    = exp(xn @ xn.T / TAU)
    top  = sum_j e[i,j] * (y_i == y_j)
    down = sum_j e[i,j]
    loss = mean(log(down) - log(top))

Strategy (cyclic-symmetric decomposition, one uniform program for all cores):
  * Host sorts rows by class (the loss is permutation invariant) so all
    same-class pairs live within a 3-tile band of the diagonal, normalizes
    rows in f32, and ships xn as bf16.
  * e is symmetric, so only half the matrix is computed: each 128-row tile
    computes a cyclic window of 33 (tiles 0-31) or 32 (tiles 32-63) j-tiles
    starting at its own diagonal. Row sums (ACT exp accumulator) give `down`
    for the tile's rows; mirror column sums of every off-diagonal 128x128
    cell - a stationary-weights matmul of the bf16 e-cell against a ones
    vector into a private [128,1] PSUM slot - give `down` for the mirrored
    pairs. top = masked row sums (DVE scalar_tensor_tensor with a host-built
    bf16 class-equality mask) plus masked mirror column sums over window
    tiles 1..2.
  * Core c owns global tiles {8k+c}; its input is pre-rotated by 128*c rows
    so one instruction stream serves all 8 cores (SPMD).
  * Everything heavy is bf16; ACT uses only the Exp table (one table load).
  * The host reassembles the per-row / per-column partial sums in f64.
"""

import sys

import numpy as np

sys.path.insert(0, "/opt/trn_rl_repo")

import ml_dtypes

TAU = 0.1
N, D = 8192, 128
P = 128
NCORES = 8
NT = N // P              # 64 global row tiles
KT = NT // NCORES        # 8 own tiles per core
BANDT = 3                # masked band tiles per window
BW = BANDT * P           # 384 mask cols per tile
MEGA = 8                 # row-tiles per transpose mega tile
NMEGA = NT // MEGA       # 8 megas
CH = 1536                # psum/exp chunk width (3 banks)
MM_N = 512               # max moving cols per matmul

_PROGRAM = None


def _win_tiles(tg):
    """window width in tiles for global tile tg (incl. diagonal tile)."""
    return 33 if tg < NT // 2 else 32


def _chunks_for_tile(k):
    """[(chunk_cols, [(psum_off, xnT_col, width), ...]), ...] for own tile k.
    Window starts at local col 1024k, wraps mod N. All 128-aligned."""
    tg_width = _win_tiles(8 * k) * P  # uniform across cores: 8k+c < 32 iff k < 4
    start = 1024 * k
    chunks = []
    done = 0
    while done < tg_width:
        cw = min(CH, tg_width - done)
        mms = []
        off = 0
        while off < cw:
            col = (start + done + off) % N
            w = min(MM_N, cw - off, N - col)
            mms.append((off, col, w))
            off += w
        chunks.append((cw, mms))
        done += cw
    return chunks


def _n_downc_slots():
    return sum(_win_tiles(8 * k) - 1 for k in range(KT))


def _build_program():
    import concourse.bacc as bacc
    import concourse.mybir as mybir
    from concourse import masks
    from concourse.tile import TileContext

    f32 = mybir.dt.float32
    bf16 = mybir.dt.bfloat16
    AF = mybir.ActivationFunctionType
    AX = mybir.AxisListType
    OP = mybir.AluOpType

    nc = bacc.Bacc("TRN2", target_bir_lowering=False)
    x_h = nc.declare_dram_parameter("x", [N, D], bf16, isOutput=False)
    m_h = nc.declare_dram_parameter("m", [P, KT * BW], bf16, isOutput=False)
    topr_h = nc.declare_dram_parameter("topr", [P, KT], f32, isOutput=True)
    downr_h = nc.declare_dram_parameter("downr", [P, 3 * KT], f32, isOutput=True)
    downc_h = nc.declare_dram_parameter(
        "downc", [P, _n_downc_slots()], f32, isOutput=True
    )
    topc_h = nc.declare_dram_parameter(
        "topc", [P, (BANDT - 1) * KT], f32, isOutput=True
    )

    # each (k, w) mirror colsum gets its own psum slot; host sums per column
    slot_of = {}
    for k in range(KT):
        for w in range(1, _win_tiles(8 * k)):
            slot_of[(k, w)] = len(slot_of)
    n_slots = len(slot_of)

    # chunk emission schedule: a chunk is ready once the megas its columns
    # (and its lhsT tile) live in have been transposed into xnT
    sched = {m: [] for m in range(NMEGA)}
    for k in range(KT):
        for j, (cw, mms) in enumerate(_chunks_for_tile(k)):
            need = {k}
            for off, col, w in mms:
                need.add(col // (MEGA * P))
                need.add((col + w - 1) // (MEGA * P))
            sched[max(need)].append((k, j, cw, mms))

    with TileContext(nc) as tc:
        with (
            tc.tile_pool(name="persist", bufs=1) as pp,
            tc.tile_pool(name="acc", bufs=1, space="PSUM") as accp,
        ):
            xnT = pp.tile([P, N], bf16)
            mt = pp.tile([P, KT * BW], bf16)
            topr = pp.tile([P, KT], f32)
            downr = pp.tile([P, 3 * KT], f32)
            acc_sb = pp.tile([P, n_slots + (BANDT - 1) * KT], f32)
            ones = pp.tile([P, 1], bf16)
            identity = pp.tile([P, P], bf16)
            acc = accp.tile([P, n_slots + (BANDT - 1) * KT], f32)

            nc.vector.memset(ones[:], 1.0)
            masks.make_identity(nc, identity[:])
            nc.scalar.dma_start(out=mt[:], in_=m_h[:, :])

            with (
                tc.tile_pool(name="xt", bufs=3) as xtp,
                tc.tile_pool(name="tp", bufs=1, space="PSUM") as tpp,
                tc.tile_pool(name="mm", bufs=2, space="PSUM") as mmp,
                tc.tile_pool(name="ep", bufs=4) as ep,
                tc.tile_pool(name="emp", bufs=2) as emp,
            ):
                pending = []  # delayed colsum emission for PE overlap

                def emit_pending():
                    for fn in pending:
                        fn()
                    pending.clear()

                def emit_mega(m):
                    xt = xtp.tile([P, MEGA, D], bf16, tag="xt", name=f"xt{m}")
                    half = MEGA // 2 * P
                    for h in range(2):
                        dma_eng = (nc.sync, nc.gpsimd, nc.scalar)[(2 * m + h) % 3]
                        dma_eng.dma_start(
                            out=xt[:, h * (MEGA // 2) : (h + 1) * (MEGA // 2)],
                            in_=x_h[
                                m * MEGA * P + h * half :
                                m * MEGA * P + (h + 1) * half,
                                :,
                            ].rearrange("(g p) d -> p g d", p=P),
                        )
                    pt = tpp.tile([P, MEGA * P], bf16, tag="pt", name=f"pt{m}")
                    for g in range(MEGA):
                        nc.tensor.transpose(
                            out=pt[:, g * P : (g + 1) * P],
                            in_=xt[:, g, :],
                            identity=identity[:],
                        )
                    nc.vector.tensor_copy(
                        out=xnT[:, m * MEGA * P : (m + 1) * MEGA * P], in_=pt[:]
                    )

                def emit_chunk(k, j, cw, mms):
                    lhsT = xnT[:, 1024 * k : 1024 * k + P]
                    ps = mmp.tile([P, CH], f32, tag="ps", name=f"ps{k}_{j}")
                    for off, col, w in mms:
                        nc.tensor.matmul(
                            out=ps[:, off : off + w],
                            lhsT=lhsT,
                            rhs=xnT[:, col : col + w],
                            start=True,
                            stop=True,
                        )
                    emit_pending()
                    e = ep.tile([P, CH], bf16, tag="e", name=f"e{k}_{j}")
                    dcol = downr[:, 3 * k + j : 3 * k + j + 1]
                    nc.scalar.activation(
                        out=e[:, :cw],
                        in_=ps[:, :cw],
                        func=AF.Exp,
                        scale=1.0 / TAU,
                        accum_out=dcol if j != 1 else None,
                    )
                    if j == 1:
                        nc.vector.tensor_reduce(
                            out=dcol, in_=e[:, :cw], axis=AX.X, op=OP.add
                        )
                    if j == 0:
                        em = emp.tile([P, BW], bf16, tag="em", name=f"em{k}")
                        nc.vector.scalar_tensor_tensor(
                            out=em[:],
                            in0=mt[:, k * BW : (k + 1) * BW],
                            scalar=1.0,
                            in1=e[:, :BW],
                            op0=OP.mult,
                            op1=OP.mult,
                            accum_out=topr[:, k : k + 1],
                        )

                        def top_cols(k=k, em=em):
                            for w in range(1, BANDT):
                                s = n_slots + (BANDT - 1) * k + w - 1
                                nc.tensor.matmul(
                                    out=acc[:, s : s + 1],
                                    lhsT=em[:, w * P : (w + 1) * P],
                                    rhs=ones[:],
                                    start=True,
                                    stop=True,
                                )

                        pending.append(top_cols)

                    def down_cols(k=k, j=j, cw=cw, e=e):
                        for wo in range(0, cw, P):
                            w = (j * CH + wo) // P
                            if w == 0:
                                continue  # diagonal tile: rows cover it
                            s = slot_of[(k, w)]
                            nc.tensor.matmul(
                                out=acc[:, s : s + 1],
                                lhsT=e[:, wo : wo + P],
                                rhs=ones[:],
                                start=True,
                                stop=True,
                            )

                    pending.append(down_cols)

                for m in range(NMEGA):
                    emit_mega(m)
                    for k, j, cw, mms in sorted(sched[m]):
                        emit_chunk(k, j, cw, mms)
                emit_pending()

            nc.vector.tensor_copy(out=acc_sb[:], in_=acc[:])
            nc.sync.dma_start(out=topr_h[:, :], in_=topr[:])
            nc.sync.dma_start(out=downr_h[:, :], in_=downr[:])
            nc.sync.dma_start(out=downc_h[:, :], in_=acc_sb[:, :n_slots])
            nc.sync.dma_start(
                out=topc_h[:, :], in_=acc_sb[:, n_slots : n_slots + (BANDT - 1) * KT]
            )
    nc.compile()
    return nc


def _get_program():
    global _PROGRAM
    if _PROGRAM is None:
        _PROGRAM = _build_program()
    return _PROGRAM


def make_in_maps(x, y):
    x = np.asarray(x, dtype=np.float32)
    y = np.asarray(y)
    perm = np.argsort(y, kind="stable")
    xs = np.ascontiguousarray(x[perm])
    xs = xs / np.linalg.norm(xs, axis=-1, keepdims=True)
    xs = xs.astype(ml_dtypes.bfloat16)
    ys = np.asarray(y)[perm].astype(np.int64)

    # class spans must fit the BANDT-tile mask band
    uniq = np.unique(ys)
    starts = np.searchsorted(ys, uniq, side="left")
    ends = np.searchsorted(ys, uniq, side="right")
    assert np.max((ends - 1) // P - starts // P) <= BANDT - 1, (
        "class span exceeds mask band; raise BANDT"
    )

    in_maps = []
    for c in range(NCORES):
        rot = P * c
        xr = np.ascontiguousarray(np.roll(xs, -rot, axis=0))
        yl = np.roll(ys, -rot)
        m = np.zeros((P, KT * BW), dtype=ml_dtypes.bfloat16)
        for k in range(KT):
            rcls = yl[1024 * k : 1024 * k + P]          # own tile k rows
            ccls = yl[(1024 * k + np.arange(BW)) % N]   # band cols
            m[:, k * BW : (k + 1) * BW] = (
                rcls[:, None] == ccls[None, :]
            ).astype(ml_dtypes.bfloat16)
        in_maps.append({"x": xr, "m": m})
    return in_maps


def finalize(results):
    """results: list of 8 dicts with topr/downr/downc/topc -> scalar loss."""
    slot_kw = []
    for k in range(KT):
        for w in range(1, _win_tiles(8 * k)):
            slot_kw.append((k, w))

    down = np.zeros(N, np.float64)
    top = np.zeros(N, np.float64)
    for c, r in enumerate(results):
        topr = np.asarray(r["topr"], np.float64)
        downr = np.asarray(r["downr"], np.float64)
        downc = np.asarray(r["downc"], np.float64)
        topc = np.asarray(r["topc"], np.float64)
        p = np.arange(P)
        for k in range(KT):
            gl = P * (8 * k + c) + p
            down[gl] += downr[:, 3 * k : 3 * k + 3].sum(axis=1)
            top[gl] += topr[:, k]
            for w in range(1, BANDT):
                vloc = 8 * k + w
                gl2 = P * ((vloc + c) % NT) + p
                top[gl2] += topc[:, (BANDT - 1) * k + w - 1]
        for s, (k, w) in enumerate(slot_kw):
            vloc = (8 * k + w) % NT
            gl = P * ((vloc + c) % NT) + p
            down[gl] += downc[:, s]
    return np.float32(np.mean(np.log(down) - np.log(top)))


def kernel(x, y):
    from concourse.bass_utils import run_bass_kernel_spmd

    nc = _get_program()
    in_maps = make_in_maps(x, y)
    res = run_bass_kernel_spmd(nc, in_maps, list(range(NCORES)))
    return finalize(res.results)


# revision 31
# speedup vs baseline: 1.0312x; 1.0312x over previous
"""Supervised contrastive loss on 8 Trainium2 NeuronCores.

Reference (N=8192, D=128, TAU=0.1, 100 classes):
    xn   = x / ||x||_row
    e    = exp(xn @ xn.T / TAU)
    top  = sum_j e[i,j] * (y_i == y_j)
    down = sum_j e[i,j]
    loss = mean(log(down) - log(top))

Strategy (cyclic-symmetric decomposition, one uniform program for all cores):
  * Host sorts rows by class (the loss is permutation invariant) so all
    same-class pairs live within a 3-tile band of the diagonal, normalizes
    rows in f32, and ships xn as bf16.
  * e is symmetric, so only half the matrix is computed: each 128-row tile
    computes a cyclic window of 33 (tiles 0-31) or 32 (tiles 32-63) j-tiles
    starting at its own diagonal. Row sums (ACT exp accumulator) give `down`
    for the tile's rows; mirror column sums of every off-diagonal 128x128
    cell - a stationary-weights matmul of the bf16 e-cell against a ones
    vector into a private [128,1] PSUM slot - give `down` for the mirrored
    pairs. top = masked row sums (DVE scalar_tensor_tensor with a host-built
    bf16 class-equality mask) plus masked mirror column sums over window
    tiles 1..2.
  * Core c owns global tiles {8k+c}; its input is pre-rotated by 128*c rows
    so one instruction stream serves all 8 cores (SPMD).
  * Everything heavy is bf16; ACT uses only the Exp table (one table load).
  * The host reassembles the per-row / per-column partial sums in f64.
"""

import sys

import numpy as np

sys.path.insert(0, "/opt/trn_rl_repo")

import ml_dtypes

TAU = 0.1
N, D = 8192, 128
P = 128
NCORES = 8
NT = N // P              # 64 global row tiles
KT = NT // NCORES        # 8 own tiles per core
BANDT = 3                # masked band tiles per window
BW = BANDT * P           # 384 mask cols per tile
MEGA = 8                 # row-tiles per transpose mega tile
NMEGA = NT // MEGA       # 8 megas
CH = 1536                # psum/exp chunk width (3 banks)
MM_N = 512               # max moving cols per matmul

_PROGRAM = None


def _win_tiles(tg):
    """window width in tiles for global tile tg (incl. diagonal tile)."""
    return 33 if tg < NT // 2 else 32


def _chunks_for_tile(k):
    """[(chunk_cols, [(psum_off, xnT_col, width), ...]), ...] for own tile k.
    Window starts at local col 1024k, wraps mod N. All 128-aligned."""
    tg_width = _win_tiles(8 * k) * P  # uniform across cores: 8k+c < 32 iff k < 4
    start = 1024 * k
    chunks = []
    done = 0
    while done < tg_width:
        cw = min(CH, tg_width - done)
        mms = []
        off = 0
        while off < cw:
            col = (start + done + off) % N
            w = min(MM_N, cw - off, N - col)
            mms.append((off, col, w))
            off += w
        chunks.append((cw, mms))
        done += cw
    return chunks


def _n_downc_slots():
    return sum(_win_tiles(8 * k) - 1 for k in range(KT))


def _build_program():
    import concourse.bacc as bacc
    import concourse.mybir as mybir
    from concourse import masks
    from concourse.tile import TileContext

    f32 = mybir.dt.float32
    bf16 = mybir.dt.bfloat16
    AF = mybir.ActivationFunctionType
    AX = mybir.AxisListType
    OP = mybir.AluOpType

    nc = bacc.Bacc("TRN2", target_bir_lowering=False)
    x_h = nc.declare_dram_parameter("x", [N, D], bf16, isOutput=False)
    m_h = nc.declare_dram_parameter("m", [P, KT * BW], bf16, isOutput=False)
    topr_h = nc.declare_dram_parameter("topr", [P, KT], f32, isOutput=True)
    downr_h = nc.declare_dram_parameter("downr", [P, 3 * KT], f32, isOutput=True)
    downc_h = nc.declare_dram_parameter(
        "downc", [P, _n_downc_slots()], f32, isOutput=True
    )
    topc_h = nc.declare_dram_parameter(
        "topc", [P, (BANDT - 1) * KT], f32, isOutput=True
    )

    # each (k, w) mirror colsum gets its own psum slot; host sums per column
    slot_of = {}
    for k in range(KT):
        for w in range(1, _win_tiles(8 * k)):
            slot_of[(k, w)] = len(slot_of)
    n_slots = len(slot_of)

    # chunk emission schedule: a chunk is ready once the megas its columns
    # (and its lhsT tile) live in have been transposed into xnT
    sched = {m: [] for m in range(NMEGA)}
    for k in range(KT):
        for j, (cw, mms) in enumerate(_chunks_for_tile(k)):
            need = {k}
            for off, col, w in mms:
                need.add(col // (MEGA * P))
                need.add((col + w - 1) // (MEGA * P))
            sched[max(need)].append((k, j, cw, mms))

    with TileContext(nc) as tc:
        with (
            tc.tile_pool(name="persist", bufs=1) as pp,
            tc.tile_pool(name="acc", bufs=1, space="PSUM") as accp,
        ):
            xnT = pp.tile([P, N], bf16)
            mt = pp.tile([P, KT * BW], bf16)
            topr = pp.tile([P, KT], f32)
            downr = pp.tile([P, 3 * KT], f32)
            acc_sb = pp.tile([P, n_slots + (BANDT - 1) * KT], f32)
            ones = pp.tile([P, 1], bf16)
            identity = pp.tile([P, P], bf16)
            acc = accp.tile([P, n_slots + (BANDT - 1) * KT], f32)

            nc.vector.memset(ones[:], 1.0)
            masks.make_identity(nc, identity[:])
            nc.scalar.dma_start(out=mt[:], in_=m_h[:, :])

            with (
                tc.tile_pool(name="xt", bufs=3) as xtp,
                tc.tile_pool(name="tp", bufs=1, space="PSUM") as tpp,
                tc.tile_pool(name="mm", bufs=2, space="PSUM") as mmp,
                tc.tile_pool(name="ep", bufs=6) as ep,
                tc.tile_pool(name="emp", bufs=3) as emp,
            ):
                # colsum batches wait two chunks before emission so the PE
                # queue always has the next gemm ahead of trailing colsums
                pending = []
                cols_done = {k: 0 for k in range(KT)}

                def emit_pending(keep=0):
                    while len(pending) > keep:
                        pending.pop(0)()

                def emit_mega(m):
                    xt = xtp.tile([P, MEGA, D], bf16, tag="xt", name=f"xt{m}")
                    qg = MEGA // 4
                    for h in range(4):
                        dma_eng = (nc.sync, nc.gpsimd, nc.scalar)[(4 * m + h) % 3]
                        dma_eng.dma_start(
                            out=xt[:, h * qg : (h + 1) * qg],
                            in_=x_h[
                                (m * MEGA + h * qg) * P :
                                (m * MEGA + (h + 1) * qg) * P,
                                :,
                            ].rearrange("(g p) d -> p g d", p=P),
                        )
                    pt = tpp.tile([P, MEGA * P], bf16, tag="pt", name=f"pt{m}")
                    for g in range(MEGA):
                        nc.tensor.transpose(
                            out=pt[:, g * P : (g + 1) * P],
                            in_=xt[:, g, :],
                            identity=identity[:],
                        )
                    nc.vector.tensor_copy(
                        out=xnT[:, m * MEGA * P : (m + 1) * MEGA * P], in_=pt[:]
                    )

                def emit_chunk(k, j, cw, mms):
                    lhsT = xnT[:, 1024 * k : 1024 * k + P]
                    ps = mmp.tile([P, CH], f32, tag="ps", name=f"ps{k}_{j}")
                    for off, col, w in mms:
                        nc.tensor.matmul(
                            out=ps[:, off : off + w],
                            lhsT=lhsT,
                            rhs=xnT[:, col : col + w],
                            start=True,
                            stop=True,
                        )
                    emit_pending(keep=1)
                    e = ep.tile([P, CH], bf16, tag="e", name=f"e{k}_{j}")
                    dcol = downr[:, 3 * k + j : 3 * k + j + 1]
                    nc.scalar.activation(
                        out=e[:, :cw],
                        in_=ps[:, :cw],
                        func=AF.Exp,
                        scale=1.0 / TAU,
                        accum_out=dcol if j != 1 else None,
                    )
                    if j == 1:
                        nc.vector.tensor_reduce(
                            out=dcol, in_=e[:, :cw], axis=AX.X, op=OP.add
                        )
                    em = None
                    if j == 0:
                        em = emp.tile([P, BW], bf16, tag="em", name=f"em{k}")
                        nc.vector.scalar_tensor_tensor(
                            out=em[:],
                            in0=mt[:, k * BW : (k + 1) * BW],
                            scalar=1.0,
                            in1=e[:, :BW],
                            op0=OP.mult,
                            op1=OP.mult,
                            accum_out=topr[:, k : k + 1],
                        )

                    def cols(k=k, j=j, cw=cw, e=e, em=em):
                        if em is not None:
                            for w in range(1, BANDT):
                                s = n_slots + (BANDT - 1) * k + w - 1
                                nc.tensor.matmul(
                                    out=acc[:, s : s + 1],
                                    lhsT=em[:, w * P : (w + 1) * P],
                                    rhs=ones[:],
                                    start=True,
                                    stop=True,
                                )
                        for wo in range(0, cw, P):
                            w = (j * CH + wo) // P
                            if w == 0:
                                continue  # diagonal tile: rows cover it
                            s = slot_of[(k, w)]
                            nc.tensor.matmul(
                                out=acc[:, s : s + 1],
                                lhsT=e[:, wo : wo + P],
                                rhs=ones[:],
                                start=True,
                                stop=True,
                            )
                        cols_done[k] += 1
                        if cols_done[k] == 3:
                            # tile k's acc slots are final: drain them so the
                            # output DMA only waits on the last tile's piece
                            s0, s1 = slot_of[(k, 1)], slot_of[(k, 1)] + _win_tiles(8 * k) - 1
                            nc.vector.tensor_copy(
                                out=acc_sb[:, s0:s1], in_=acc[:, s0:s1]
                            )
                            t0 = n_slots + (BANDT - 1) * k
                            nc.vector.tensor_copy(
                                out=acc_sb[:, t0 : t0 + BANDT - 1],
                                in_=acc[:, t0 : t0 + BANDT - 1],
                            )

                    pending.append(cols)

                for m in range(NMEGA):
                    emit_mega(m)
                    for k, j, cw, mms in sorted(sched[m]):
                        emit_chunk(k, j, cw, mms)
                emit_pending()

            nc.sync.dma_start(out=topr_h[:, :], in_=topr[:])
            nc.sync.dma_start(out=downr_h[:, :], in_=downr[:])
            nc.sync.dma_start(out=downc_h[:, :], in_=acc_sb[:, :n_slots])
            nc.sync.dma_start(
                out=topc_h[:, :], in_=acc_sb[:, n_slots : n_slots + (BANDT - 1) * KT]
            )
    nc.compile()
    return nc


def _get_program():
    global _PROGRAM
    if _PROGRAM is None:
        _PROGRAM = _build_program()
    return _PROGRAM


def make_in_maps(x, y):
    x = np.asarray(x, dtype=np.float32)
    y = np.asarray(y)
    perm = np.argsort(y, kind="stable")
    xs = np.ascontiguousarray(x[perm])
    xs = xs / np.linalg.norm(xs, axis=-1, keepdims=True)
    xs = xs.astype(ml_dtypes.bfloat16)
    ys = np.asarray(y)[perm].astype(np.int64)

    # class spans must fit the BANDT-tile mask band
    uniq = np.unique(ys)
    starts = np.searchsorted(ys, uniq, side="left")
    ends = np.searchsorted(ys, uniq, side="right")
    assert np.max((ends - 1) // P - starts // P) <= BANDT - 1, (
        "class span exceeds mask band; raise BANDT"
    )

    in_maps = []
    for c in range(NCORES):
        rot = P * c
        xr = np.ascontiguousarray(np.roll(xs, -rot, axis=0))
        yl = np.roll(ys, -rot)
        m = np.zeros((P, KT * BW), dtype=ml_dtypes.bfloat16)
        for k in range(KT):
            rcls = yl[1024 * k : 1024 * k + P]          # own tile k rows
            ccls = yl[(1024 * k + np.arange(BW)) % N]   # band cols
            m[:, k * BW : (k + 1) * BW] = (
                rcls[:, None] == ccls[None, :]
            ).astype(ml_dtypes.bfloat16)
        in_maps.append({"x": xr, "m": m})
    return in_maps


def finalize(results):
    """results: list of 8 dicts with topr/downr/downc/topc -> scalar loss."""
    slot_kw = []
    for k in range(KT):
        for w in range(1, _win_tiles(8 * k)):
            slot_kw.append((k, w))

    down = np.zeros(N, np.float64)
    top = np.zeros(N, np.float64)
    for c, r in enumerate(results):
        topr = np.asarray(r["topr"], np.float64)
        downr = np.asarray(r["downr"], np.float64)
        downc = np.asarray(r["downc"], np.float64)
        topc = np.asarray(r["topc"], np.float64)
        p = np.arange(P)
        for k in range(KT):
            gl = P * (8 * k + c) + p
            down[gl] += downr[:, 3 * k : 3 * k + 3].sum(axis=1)
            top[gl] += topr[:, k]
            for w in range(1, BANDT):
                vloc = 8 * k + w
                gl2 = P * ((vloc + c) % NT) + p
                top[gl2] += topc[:, (BANDT - 1) * k + w - 1]
        for s, (k, w) in enumerate(slot_kw):
            vloc = (8 * k + w) % NT
            gl = P * ((vloc + c) % NT) + p
            down[gl] += downc[:, s]
    return np.float32(np.mean(np.log(down) - np.log(top)))


def kernel(x, y):
    from concourse.bass_utils import run_bass_kernel_spmd

    nc = _get_program()
    in_maps = make_in_maps(x, y)
    res = run_bass_kernel_spmd(nc, in_maps, list(range(NCORES)))
    return finalize(res.results)


# revision 38
# speedup vs baseline: 1.1973x; 1.1611x over previous
"""Supervised contrastive loss on 8 Trainium2 NeuronCores.

Reference (N=8192, D=128, TAU=0.1, 100 classes):
    xn   = x / ||x||_row
    e    = exp(xn @ xn.T / TAU)
    top  = sum_j e[i,j] * (y_i == y_j)
    down = sum_j e[i,j]
    loss = mean(log(down) - log(top))

Strategy (cyclic-symmetric decomposition, one uniform program for all cores):
  * Host sorts rows by class (the loss is permutation invariant) so all
    same-class pairs live within a 3-tile band of the diagonal, normalizes
    rows in f32, and ships xn as bf16.
  * e is symmetric, so only half the matrix is computed: each 128-row tile
    computes a cyclic window of 33 (tiles 0-31) or 32 (tiles 32-63) j-tiles
    starting at its own diagonal. Row sums (ACT exp accumulator) give `down`
    for the tile's rows; mirror column sums of every off-diagonal 128x128
    cell - a stationary-weights matmul of the bf16 e-cell against a ones
    vector into a private [128,1] PSUM slot - give `down` for the mirrored
    pairs. top = masked row sums (DVE scalar_tensor_tensor with a host-built
    bf16 class-equality mask) plus masked mirror column sums over window
    tiles 1..2.
  * Core c owns global tiles {8k+c}; its input is pre-rotated by 128*c rows
    so one instruction stream serves all 8 cores (SPMD).
  * Everything heavy is bf16; ACT uses only the Exp table (one table load).
  * The host reassembles the per-row / per-column partial sums in f64.
"""

import sys

import numpy as np

sys.path.insert(0, "/opt/trn_rl_repo")

import ml_dtypes

TAU = 0.1
N, D = 8192, 128
P = 128
NCORES = 8
NT = N // P              # 64 global row tiles
KT = NT // NCORES        # 8 own tiles per core
BANDT = 3                # masked band tiles per window
BW = BANDT * P           # 384 mask cols per tile
MEGA = 8                 # row-tiles per transpose mega tile
NMEGA = NT // MEGA       # 8 megas
CH = 1536                # psum/exp chunk width (3 banks)
MM_N = 512               # max moving cols per matmul

_PROGRAM = None


def _win_tiles(tg):
    """window width in tiles for global tile tg (incl. diagonal tile)."""
    return 33 if tg < NT // 2 else 32


def _chunks_for_tile(k):
    """[(chunk_cols, [(psum_off, xnT_col, width), ...]), ...] for own tile k.
    Window starts at local col 1024k, wraps mod N. All 128-aligned."""
    tg_width = _win_tiles(8 * k) * P  # uniform across cores: 8k+c < 32 iff k < 4
    start = 1024 * k
    chunks = []
    done = 0
    while done < tg_width:
        cw = min(CH, tg_width - done)
        mms = []
        off = 0
        while off < cw:
            col = (start + done + off) % N
            w = min(MM_N, cw - off, N - col)
            mms.append((off, col, w))
            off += w
        chunks.append((cw, mms))
        done += cw
    return chunks


def _n_downc_slots():
    return sum(_win_tiles(8 * k) - 1 for k in range(KT))


def _build_program():
    import concourse.bacc as bacc
    import concourse.mybir as mybir
    from concourse.tile import TileContext

    f32 = mybir.dt.float32
    bf16 = mybir.dt.bfloat16
    AF = mybir.ActivationFunctionType
    AX = mybir.AxisListType
    OP = mybir.AluOpType

    nc = bacc.Bacc("TRN2", target_bir_lowering=False)
    x_h = nc.declare_dram_parameter("xnt", [P, N], bf16, isOutput=False)
    m_h = nc.declare_dram_parameter("m", [P, KT * BW], bf16, isOutput=False)
    topr_h = nc.declare_dram_parameter("topr", [P, KT], f32, isOutput=True)
    downr_h = nc.declare_dram_parameter("downr", [P, 3 * KT], f32, isOutput=True)
    downc_h = nc.declare_dram_parameter(
        "downc", [P, _n_downc_slots()], f32, isOutput=True
    )
    topc_h = nc.declare_dram_parameter(
        "topc", [P, (BANDT - 1) * KT], f32, isOutput=True
    )

    # each (k, w) mirror colsum gets its own psum slot; host sums per column
    slot_of = {}
    for k in range(KT):
        for w in range(1, _win_tiles(8 * k)):
            slot_of[(k, w)] = len(slot_of)
    n_slots = len(slot_of)

    with TileContext(nc) as tc:
        with (
            tc.tile_pool(name="persist", bufs=1) as pp,
            tc.tile_pool(name="acc", bufs=1, space="PSUM") as accp,
        ):
            xnT = pp.tile([P, N], bf16)
            mt = pp.tile([P, KT * BW], bf16)
            topr = pp.tile([P, KT], f32)
            downr = pp.tile([P, 3 * KT], f32)
            acc_sb = pp.tile([P, n_slots + (BANDT - 1) * KT], f32)
            ones = pp.tile([P, 1], bf16)
            acc = accp.tile([P, n_slots + (BANDT - 1) * KT], f32)

            nc.vector.memset(ones[:], 1.0)
            nc.scalar.dma_start(out=mt[:], in_=m_h[:, :])
            # xnT arrives pre-transposed: 16KB contiguous per partition,
            # split across the DMA queues so the first slice lands fast
            NQ = 4
            for h in range(NQ):
                dma_eng = (nc.sync, nc.gpsimd)[h % 2]
                dma_eng.dma_start(
                    out=xnT[:, h * N // NQ : (h + 1) * N // NQ],
                    in_=x_h[:, h * N // NQ : (h + 1) * N // NQ],
                )

            with (
                tc.tile_pool(name="mm", bufs=2, space="PSUM") as mmp,
                tc.tile_pool(name="ep", bufs=6) as ep,
                tc.tile_pool(name="emp", bufs=3) as emp,
            ):
                # colsum batches wait two chunks before emission so the PE
                # queue always has the next gemm ahead of trailing colsums
                pending = []
                cols_done = {k: 0 for k in range(KT)}

                def emit_pending(keep=0):
                    while len(pending) > keep:
                        pending.pop(0)()

                def emit_chunk(k, j, cw, mms):
                    lhsT = xnT[:, 1024 * k : 1024 * k + P]
                    ps = mmp.tile([P, CH], f32, tag="ps", name=f"ps{k}_{j}")
                    for off, col, w in mms:
                        nc.tensor.matmul(
                            out=ps[:, off : off + w],
                            lhsT=lhsT,
                            rhs=xnT[:, col : col + w],
                            start=True,
                            stop=True,
                        )
                    emit_pending(keep=1)
                    e = ep.tile([P, CH], bf16, tag="e", name=f"e{k}_{j}")
                    dcol = downr[:, 3 * k + j : 3 * k + j + 1]
                    nc.scalar.activation(
                        out=e[:, :cw],
                        in_=ps[:, :cw],
                        func=AF.Exp,
                        scale=1.0 / TAU,
                        accum_out=dcol if j != 1 else None,
                    )
                    if j == 1:
                        nc.vector.tensor_reduce(
                            out=dcol, in_=e[:, :cw], axis=AX.X, op=OP.add
                        )
                    em = None
                    if j == 0:
                        em = emp.tile([P, BW], bf16, tag="em", name=f"em{k}")
                        nc.vector.scalar_tensor_tensor(
                            out=em[:],
                            in0=mt[:, k * BW : (k + 1) * BW],
                            scalar=1.0,
                            in1=e[:, :BW],
                            op0=OP.mult,
                            op1=OP.mult,
                            accum_out=topr[:, k : k + 1],
                        )

                    def cols(k=k, j=j, cw=cw, e=e, em=em):
                        if em is not None:
                            for w in range(1, BANDT):
                                s = n_slots + (BANDT - 1) * k + w - 1
                                nc.tensor.matmul(
                                    out=acc[:, s : s + 1],
                                    lhsT=em[:, w * P : (w + 1) * P],
                                    rhs=ones[:],
                                    start=True,
                                    stop=True,
                                )
                        for wo in range(0, cw, P):
                            w = (j * CH + wo) // P
                            if w == 0:
                                continue  # diagonal tile: rows cover it
                            s = slot_of[(k, w)]
                            nc.tensor.matmul(
                                out=acc[:, s : s + 1],
                                lhsT=e[:, wo : wo + P],
                                rhs=ones[:],
                                start=True,
                                stop=True,
                            )
                        cols_done[k] += 1
                        if cols_done[k] == 3:
                            # tile k's acc slots are final: drain them so the
                            # output DMA only waits on the last tile's piece
                            s0, s1 = slot_of[(k, 1)], slot_of[(k, 1)] + _win_tiles(8 * k) - 1
                            nc.vector.tensor_copy(
                                out=acc_sb[:, s0:s1], in_=acc[:, s0:s1]
                            )
                            t0 = n_slots + (BANDT - 1) * k
                            nc.vector.tensor_copy(
                                out=acc_sb[:, t0 : t0 + BANDT - 1],
                                in_=acc[:, t0 : t0 + BANDT - 1],
                            )

                    pending.append(cols)

                for k in range(KT):
                    for j, (cw, mms) in enumerate(_chunks_for_tile(k)):
                        emit_chunk(k, j, cw, mms)
                emit_pending()

            nc.sync.dma_start(out=topr_h[:, :], in_=topr[:])
            nc.sync.dma_start(out=downr_h[:, :], in_=downr[:])
            nc.sync.dma_start(out=downc_h[:, :], in_=acc_sb[:, :n_slots])
            nc.sync.dma_start(
                out=topc_h[:, :], in_=acc_sb[:, n_slots : n_slots + (BANDT - 1) * KT]
            )
    nc.compile()
    return nc


def _get_program():
    global _PROGRAM
    if _PROGRAM is None:
        _PROGRAM = _build_program()
    return _PROGRAM


def make_in_maps(x, y):
    x = np.asarray(x, dtype=np.float32)
    y = np.asarray(y)
    perm = np.argsort(y, kind="stable")
    xs = np.ascontiguousarray(x[perm])
    xs = xs / np.linalg.norm(xs, axis=-1, keepdims=True)
    xs = xs.astype(ml_dtypes.bfloat16)
    ys = np.asarray(y)[perm].astype(np.int64)

    # class spans must fit the BANDT-tile mask band
    uniq = np.unique(ys)
    starts = np.searchsorted(ys, uniq, side="left")
    ends = np.searchsorted(ys, uniq, side="right")
    assert np.max((ends - 1) // P - starts // P) <= BANDT - 1, (
        "class span exceeds mask band; raise BANDT"
    )

    in_maps = []
    for c in range(NCORES):
        rot = P * c
        xr = np.ascontiguousarray(np.roll(xs, -rot, axis=0))
        yl = np.roll(ys, -rot)
        m = np.zeros((P, KT * BW), dtype=ml_dtypes.bfloat16)
        for k in range(KT):
            rcls = yl[1024 * k : 1024 * k + P]          # own tile k rows
            ccls = yl[(1024 * k + np.arange(BW)) % N]   # band cols
            m[:, k * BW : (k + 1) * BW] = (
                rcls[:, None] == ccls[None, :]
            ).astype(ml_dtypes.bfloat16)
        in_maps.append({"xnt": np.ascontiguousarray(xr.T), "m": m})
    return in_maps


def finalize(results):
    """results: list of 8 dicts with topr/downr/downc/topc -> scalar loss."""
    slot_kw = []
    for k in range(KT):
        for w in range(1, _win_tiles(8 * k)):
            slot_kw.append((k, w))

    down = np.zeros(N, np.float64)
    top = np.zeros(N, np.float64)
    for c, r in enumerate(results):
        topr = np.asarray(r["topr"], np.float64)
        downr = np.asarray(r["downr"], np.float64)
        downc = np.asarray(r["downc"], np.float64)
        topc = np.asarray(r["topc"], np.float64)
        p = np.arange(P)
        for k in range(KT):
            gl = P * (8 * k + c) + p
            down[gl] += downr[:, 3 * k : 3 * k + 3].sum(axis=1)
            top[gl] += topr[:, k]
            for w in range(1, BANDT):
                vloc = 8 * k + w
                gl2 = P * ((vloc + c) % NT) + p
                top[gl2] += topc[:, (BANDT - 1) * k + w - 1]
        for s, (k, w) in enumerate(slot_kw):
            vloc = (8 * k + w) % NT
            gl = P * ((vloc + c) % NT) + p
            down[gl] += downc[:, s]
    return np.float32(np.mean(np.log(down) - np.log(top)))


def kernel(x, y):
    from concourse.bass_utils import run_bass_kernel_spmd

    nc = _get_program()
    in_maps = make_in_maps(x, y)
    res = run_bass_kernel_spmd(nc, in_maps, list(range(NCORES)))
    return finalize(res.results)


# revision 39
# speedup vs baseline: 1.2386x; 1.0345x over previous
"""Supervised contrastive loss on 8 Trainium2 NeuronCores.

Reference (N=8192, D=128, TAU=0.1, 100 classes):
    xn   = x / ||x||_row
    e    = exp(xn @ xn.T / TAU)
    top  = sum_j e[i,j] * (y_i == y_j)
    down = sum_j e[i,j]
    loss = mean(log(down) - log(top))

Strategy (cyclic-symmetric decomposition, one uniform program for all cores):
  * Host sorts rows by class (the loss is permutation invariant) so all
    same-class pairs live within a 3-tile band of the diagonal, normalizes
    rows in f32, and ships xn as bf16.
  * e is symmetric, so only half the matrix is computed: each 128-row tile
    computes a cyclic window of 33 (tiles 0-31) or 32 (tiles 32-63) j-tiles
    starting at its own diagonal. Row sums (ACT exp accumulator) give `down`
    for the tile's rows; mirror column sums of every off-diagonal 128x128
    cell - a stationary-weights matmul of the bf16 e-cell against a ones
    vector into a private [128,1] PSUM slot - give `down` for the mirrored
    pairs. top = masked row sums (DVE scalar_tensor_tensor with a host-built
    bf16 class-equality mask) plus masked mirror column sums over window
    tiles 1..2.
  * Core c owns global tiles {8k+c}; its input is pre-rotated by 128*c rows
    so one instruction stream serves all 8 cores (SPMD).
  * Everything heavy is bf16; ACT uses only the Exp table (one table load).
  * The host reassembles the per-row / per-column partial sums in f64.
"""

import sys

import numpy as np

sys.path.insert(0, "/opt/trn_rl_repo")

import ml_dtypes

TAU = 0.1
N, D = 8192, 128
P = 128
NCORES = 8
NT = N // P              # 64 global row tiles
KT = NT // NCORES        # 8 own tiles per core
BANDT = 3                # masked band tiles per window
BW = BANDT * P           # 384 mask cols per tile
MEGA = 8                 # row-tiles per transpose mega tile
NMEGA = NT // MEGA       # 8 megas
CH = 1536                # psum/exp chunk width (3 banks)
MM_N = 512               # max moving cols per matmul

_PROGRAM = None


def _win_tiles(tg):
    """window width in tiles for global tile tg (incl. diagonal tile)."""
    return 33 if tg < NT // 2 else 32


def _chunks_for_tile(k):
    """[(chunk_cols, [(psum_off, xnT_col, width), ...]), ...] for own tile k.
    Window starts at local col 1024k, wraps mod N. All 128-aligned."""
    tg_width = _win_tiles(8 * k) * P  # uniform across cores: 8k+c < 32 iff k < 4
    start = 1024 * k
    chunks = []
    done = 0
    while done < tg_width:
        cw = min(CH, tg_width - done)
        mms = []
        off = 0
        while off < cw:
            col = (start + done + off) % N
            w = min(MM_N, cw - off, N - col)
            mms.append((off, col, w))
            off += w
        chunks.append((cw, mms))
        done += cw
    return chunks


def _n_downc_slots():
    return sum(_win_tiles(8 * k) - 1 for k in range(KT))


def _build_program():
    import concourse.bacc as bacc
    import concourse.mybir as mybir
    from concourse.tile import TileContext

    f32 = mybir.dt.float32
    bf16 = mybir.dt.bfloat16
    AF = mybir.ActivationFunctionType
    AX = mybir.AxisListType
    OP = mybir.AluOpType

    nc = bacc.Bacc("TRN2", target_bir_lowering=False)
    x_h = nc.declare_dram_parameter("xnt", [P, N], bf16, isOutput=False)
    m_h = nc.declare_dram_parameter("m", [P, KT * BW], bf16, isOutput=False)
    topr_h = nc.declare_dram_parameter("topr", [P, KT], f32, isOutput=True)
    downr_h = nc.declare_dram_parameter("downr", [P, 3 * KT], f32, isOutput=True)
    downc_h = nc.declare_dram_parameter(
        "downc", [P, _n_downc_slots()], f32, isOutput=True
    )
    topc_h = nc.declare_dram_parameter(
        "topc", [P, (BANDT - 1) * KT], f32, isOutput=True
    )

    # each (k, w) mirror colsum gets its own psum slot; host sums per column
    slot_of = {}
    for k in range(KT):
        for w in range(1, _win_tiles(8 * k)):
            slot_of[(k, w)] = len(slot_of)
    n_slots = len(slot_of)

    with TileContext(nc) as tc:
        with (
            tc.tile_pool(name="persist", bufs=1) as pp,
            tc.tile_pool(name="acc", bufs=1, space="PSUM") as accp,
        ):
            xnT = pp.tile([P, N], bf16)
            mt = pp.tile([P, KT * BW], bf16)
            topr = pp.tile([P, KT], f32)
            downr = pp.tile([P, 3 * KT], f32)
            acc_sb = pp.tile([P, n_slots + (BANDT - 1) * KT], f32)
            ones = pp.tile([P, 1], bf16)
            acc = accp.tile([P, n_slots + (BANDT - 1) * KT], f32)

            nc.vector.memset(ones[:], 1.0)
            nc.scalar.dma_start(out=mt[:], in_=m_h[:, :])
            # xnT arrives pre-transposed: 16KB contiguous per partition,
            # split across the DMA queues so the first slice lands fast
            NQ = 8
            for h in range(NQ):
                dma_eng = (nc.sync, nc.gpsimd)[h % 2]
                dma_eng.dma_start(
                    out=xnT[:, h * N // NQ : (h + 1) * N // NQ],
                    in_=x_h[:, h * N // NQ : (h + 1) * N // NQ],
                )

            with (
                tc.tile_pool(name="mm", bufs=2, space="PSUM") as mmp,
                tc.tile_pool(name="ep", bufs=6) as ep,
                tc.tile_pool(name="emp", bufs=3) as emp,
            ):
                # colsum batches wait two chunks before emission so the PE
                # queue always has the next gemm ahead of trailing colsums
                pending = []
                cols_done = {k: 0 for k in range(KT)}

                def emit_pending(keep=0):
                    while len(pending) > keep:
                        pending.pop(0)()

                def emit_chunk(k, j, cw, mms):
                    lhsT = xnT[:, 1024 * k : 1024 * k + P]
                    ps = mmp.tile([P, CH], f32, tag="ps", name=f"ps{k}_{j}")
                    for off, col, w in mms:
                        nc.tensor.matmul(
                            out=ps[:, off : off + w],
                            lhsT=lhsT,
                            rhs=xnT[:, col : col + w],
                            start=True,
                            stop=True,
                        )
                    emit_pending(keep=1)
                    e = ep.tile([P, CH], bf16, tag="e", name=f"e{k}_{j}")
                    dcol = downr[:, 3 * k + j : 3 * k + j + 1]
                    nc.scalar.activation(
                        out=e[:, :cw],
                        in_=ps[:, :cw],
                        func=AF.Exp,
                        scale=1.0 / TAU,
                        accum_out=dcol if j != 1 else None,
                    )
                    if j == 1:
                        nc.vector.tensor_reduce(
                            out=dcol, in_=e[:, :cw], axis=AX.X, op=OP.add
                        )
                    em = None
                    if j == 0:
                        em = emp.tile([P, BW], bf16, tag="em", name=f"em{k}")
                        nc.vector.scalar_tensor_tensor(
                            out=em[:],
                            in0=mt[:, k * BW : (k + 1) * BW],
                            scalar=1.0,
                            in1=e[:, :BW],
                            op0=OP.mult,
                            op1=OP.mult,
                            accum_out=topr[:, k : k + 1],
                        )

                    def cols(k=k, j=j, cw=cw, e=e, em=em):
                        if em is not None:
                            for w in range(1, BANDT):
                                s = n_slots + (BANDT - 1) * k + w - 1
                                nc.tensor.matmul(
                                    out=acc[:, s : s + 1],
                                    lhsT=em[:, w * P : (w + 1) * P],
                                    rhs=ones[:],
                                    start=True,
                                    stop=True,
                                )
                        for wo in range(0, cw, P):
                            w = (j * CH + wo) // P
                            if w == 0:
                                continue  # diagonal tile: rows cover it
                            s = slot_of[(k, w)]
                            nc.tensor.matmul(
                                out=acc[:, s : s + 1],
                                lhsT=e[:, wo : wo + P],
                                rhs=ones[:],
                                start=True,
                                stop=True,
                            )
                        cols_done[k] += 1
                        if cols_done[k] == 3:
                            # tile k's acc slots are final: drain them so the
                            # output DMA only waits on the last tile's piece
                            s0, s1 = slot_of[(k, 1)], slot_of[(k, 1)] + _win_tiles(8 * k) - 1
                            nc.vector.tensor_copy(
                                out=acc_sb[:, s0:s1], in_=acc[:, s0:s1]
                            )
                            t0 = n_slots + (BANDT - 1) * k
                            nc.vector.tensor_copy(
                                out=acc_sb[:, t0 : t0 + BANDT - 1],
                                in_=acc[:, t0 : t0 + BANDT - 1],
                            )

                    pending.append(cols)

                for k in range(KT):
                    for j, (cw, mms) in enumerate(_chunks_for_tile(k)):
                        emit_chunk(k, j, cw, mms)
                emit_pending()

            nc.sync.dma_start(out=topr_h[:, :], in_=topr[:])
            nc.sync.dma_start(out=downr_h[:, :], in_=downr[:])
            nc.sync.dma_start(out=downc_h[:, :], in_=acc_sb[:, :n_slots])
            nc.sync.dma_start(
                out=topc_h[:, :], in_=acc_sb[:, n_slots : n_slots + (BANDT - 1) * KT]
            )
    nc.compile()
    return nc


def _get_program():
    global _PROGRAM
    if _PROGRAM is None:
        _PROGRAM = _build_program()
    return _PROGRAM


def make_in_maps(x, y):
    x = np.asarray(x, dtype=np.float32)
    y = np.asarray(y)
    perm = np.argsort(y, kind="stable")
    xs = np.ascontiguousarray(x[perm])
    xs = xs / np.linalg.norm(xs, axis=-1, keepdims=True)
    xs = xs.astype(ml_dtypes.bfloat16)
    ys = np.asarray(y)[perm].astype(np.int64)

    # class spans must fit the BANDT-tile mask band
    uniq = np.unique(ys)
    starts = np.searchsorted(ys, uniq, side="left")
    ends = np.searchsorted(ys, uniq, side="right")
    assert np.max((ends - 1) // P - starts // P) <= BANDT - 1, (
        "class span exceeds mask band; raise BANDT"
    )

    in_maps = []
    for c in range(NCORES):
        rot = P * c
        xr = np.ascontiguousarray(np.roll(xs, -rot, axis=0))
        yl = np.roll(ys, -rot)
        m = np.zeros((P, KT * BW), dtype=ml_dtypes.bfloat16)
        for k in range(KT):
            rcls = yl[1024 * k : 1024 * k + P]          # own tile k rows
            ccls = yl[(1024 * k + np.arange(BW)) % N]   # band cols
            m[:, k * BW : (k + 1) * BW] = (
                rcls[:, None] == ccls[None, :]
            ).astype(ml_dtypes.bfloat16)
        in_maps.append({"xnt": np.ascontiguousarray(xr.T), "m": m})
    return in_maps


def finalize(results):
    """results: list of 8 dicts with topr/downr/downc/topc -> scalar loss."""
    slot_kw = []
    for k in range(KT):
        for w in range(1, _win_tiles(8 * k)):
            slot_kw.append((k, w))

    down = np.zeros(N, np.float64)
    top = np.zeros(N, np.float64)
    for c, r in enumerate(results):
        topr = np.asarray(r["topr"], np.float64)
        downr = np.asarray(r["downr"], np.float64)
        downc = np.asarray(r["downc"], np.float64)
        topc = np.asarray(r["topc"], np.float64)
        p = np.arange(P)
        for k in range(KT):
            gl = P * (8 * k + c) + p
            down[gl] += downr[:, 3 * k : 3 * k + 3].sum(axis=1)
            top[gl] += topr[:, k]
            for w in range(1, BANDT):
                vloc = 8 * k + w
                gl2 = P * ((vloc + c) % NT) + p
                top[gl2] += topc[:, (BANDT - 1) * k + w - 1]
        for s, (k, w) in enumerate(slot_kw):
            vloc = (8 * k + w) % NT
            gl = P * ((vloc + c) % NT) + p
            down[gl] += downc[:, s]
    return np.float32(np.mean(np.log(down) - np.log(top)))


def kernel(x, y):
    from concourse.bass_utils import run_bass_kernel_spmd

    nc = _get_program()
    in_maps = make_in_maps(x, y)
    res = run_bass_kernel_spmd(nc, in_maps, list(range(NCORES)))
    return finalize(res.results)
